# revision 40
# baseline (speedup 1.0000x reference)
"""Trainium2 Bass kernel for multi-head attention (b=4, n=2048, d=512, h=8, dk=dv=64).

Sharding: 8 cores = 4 batches x 2 query-halves. Each core computes K/V for its
full batch sequence (2048) and attention outputs for its 1024 query rows.
No collectives needed; host stacks the per-core [1024, 512] outputs.

Per-core dataflow (all matmul operands fp16 except P/V in bf16; fp16 keeps
f32r-class precision - 10 mantissa bits - while its 2-byte LDWEIGHTS streams
at full PE rate, where 4-byte f32r stationary loads stall the PE ~130ns per
matmul):
  x^T [512, 2048] staged in SBUF as fp16.
  Head-PAIR packed projections: Q^T/K^T per pair p (heads 2p, 2p+1):
    lhsT = w[:, ch, p*128:(p+1)*128] -> out [128 rows = headA 64 dims | headB
    64 dims, n].  Rel-bias is folded into Q^T via a per-partition scalar add.
  K^T stored per head in [128, 2048] fp16 tiles with the OTHER head's
    partition half zeroed, so every ST matmul is a uniform 128x128 tile
    config (mixed 64-row/128-row configs cost ~200ns per switch):
    lhsT = kt_h[h][:, jc*128:+128], rhs = qt [128, 512i] -> S^T [128j, 512i].
    The zero half multiplies the other head's qt rows to zero contribution.
  V = x Wv (+ ones col) [per j-chunk: 128j, 8h*65] in bf16.
  P^T = exp(S^T) -> bf16 (no max-subtraction: logits < ~50; bf16 range ok).
    One ACT instruction per 2 j-chunks ([128, 1024], from a 2-bank PSUM st
    tile) amortizes the ~260ns fixed Activation-engine cost.
  PV (bf16) accumulated over 16 j-chunks into [65, 512] PSUM per (head,
    i-half); row 64 = denominator (ones column of V_aug).  PV lags ST by 3
    batches so the exp semaphore is always satisfied when PV issues
    (lockstep ST->exp->PV coupling otherwise costs ~190ns/batch).
  Normalization: reciprocal_approx_fast + gpsimd partition broadcast + DVE
    multiply -> outt (fp16, one tile per head pair so the output projection
    only waits on the pairs it reads).
  y = outt^T @ Wo + bo, accumulated over head pairs; the first i-half is
    projected during head 7's second half to shorten the tail.

Schedule: head h's Q/K projections are emitted during head h-2 (pair-ahead);
V projections and pair-0's K groups 2-3 are interleaved into head 0's ST
stream so the PE never waits on late x/wv DMA chunks.  Input DMAs are
priority-ordered (wq, x[own queries], wk, wv, x[rest], wo) across the
sync/gpsimd/scalar queues.

PSUM budget (8 banks): st pool 2 bufs x [128, 2jc*512] f32 (2 banks each)
+ qk pool 2 bufs x [128, 512] (1 bank each) + pv 2 bufs x [65, 512] (1 bank).
"""
import numpy as np

B, N, MODEL = 4, 2048, 512
H, DK = 8, 64
SCALE = DK ** -0.5
NP = H // 2         # head pairs
NI = 1024           # query rows per core
NCH = MODEL // 128  # model-dim chunks
NJC = N // 128      # key/value chunks
JB = 2              # j-chunks per ST/exp batch
NB = NJC // JB      # batches per (head, i-half)

_COMPILED = None


def _build():
    import concourse.bass as bass
    from concourse import bacc
    import concourse.mybir as mybir
    import concourse.tile as tile

    F32 = mybir.dt.float32
    F32R = mybir.dt.float32r
    BF16 = mybir.dt.bfloat16
    FP16 = mybir.dt.float16
    EXP = mybir.ActivationFunctionType.Exp

    nc = bacc.Bacc("TRN2", target_bir_lowering=False, debug=False, num_devices=8)
    xt_in = nc.dram_tensor("xt", [MODEL, N], FP16, kind="ExternalInput")
    wq_in = nc.dram_tensor("wq", [MODEL, MODEL], FP16, kind="ExternalInput")
    wk_in = nc.dram_tensor("wk", [MODEL, MODEL], FP16, kind="ExternalInput")
    wv_in = nc.dram_tensor("wv", [MODEL, MODEL], FP16, kind="ExternalInput")
    relb_in = nc.dram_tensor("relb", [128, NP], F32, kind="ExternalInput")
    wo_in = nc.dram_tensor("wo", [MODEL, MODEL], FP16, kind="ExternalInput")
    bo_in = nc.dram_tensor("bo", [1, MODEL], F32, kind="ExternalInput")
    onesb_in = nc.dram_tensor("onesb", [128, NJC * H], BF16, kind="ExternalInput")
    y_out = nc.dram_tensor("y", [NI, MODEL], F32, kind="ExternalOutput")

    with tile.TileContext(nc) as tc:
        with (
            tc.tile_pool(name="w", bufs=1) as wp,
            tc.tile_pool(name="acts", bufs=1) as ap,
            tc.tile_pool(name="st", bufs=2, space="PSUM") as stp,
            tc.tile_pool(name="qk", bufs=2, space="PSUM") as qkp,
            tc.tile_pool(name="pv", bufs=2, space="PSUM") as pvp,
        ):
            # ---------- persistent tiles ----------
            wq_c = [wp.tile([128, MODEL], FP16, name=f"wq{i}", tag=f"wq{i}")
                    for i in range(NCH)]
            wk = wp.tile([128, NCH, MODEL], FP16, tag="wk")
            wv = wp.tile([128, NCH, MODEL], FP16, tag="wv")
            wo = wp.tile([128, NP, MODEL], FP16, tag="wo")
            relb = wp.tile([128, NP], F32, tag="relb")
            bo = wp.tile([1, MODEL], F32, tag="bo")
            bo_b = wp.tile([128, MODEL], F32, tag="bo_b")
            onesb_t = wp.tile([128, NJC * H], BF16, tag="onesb")

            xt0 = ap.tile([128, NCH, 512], FP16, tag="xt0")
            xt1 = ap.tile([128, NCH, 512], FP16, tag="xt1")
            xt2 = ap.tile([128, NCH, 512], FP16, tag="xt2")
            xt3 = ap.tile([128, NCH, 512], FP16, tag="xt3")
            xts = [xt0, xt1, xt2, xt3]
            vv_a = ap.tile([128, NJC // 2, H * 65], BF16, tag="vva")
            vv_b = ap.tile([128, NJC // 2, H * 65], BF16, tag="vvb")
            qt_t0 = ap.tile([128, NI], FP16, tag="qt0")
            qt_t1 = ap.tile([128, NI], FP16, tag="qt1")
            kt_h = [ap.tile([128, NJC * 128], FP16, name=f"kt_h{i}",
                            tag=f"kt{i}") for i in range(4)]
            outt_p = [ap.tile([128, NI], FP16, name=f"outt{i}", tag=f"outt{i}")
                      for i in range(NP)]

            def vvt(jc):
                return (vv_a if jc < NJC // 2 else vv_b)[:, jc % (NJC // 2)]

            def r3(d):
                return d[:].rearrange("(c p) n -> p c n", p=128)

            def xtv(ch, start, size):
                t = xts[start // 512]
                off = start % 512
                assert off + size <= 512
                return t[:, ch, off:off + size]

            # ---------- input staging, priority order ----------
            dma_engs = (nc.sync, nc.gpsimd, nc.scalar)
            _dma_i = [0]

            def dma(out, in_):
                dma_engs[_dma_i[0] % len(dma_engs)].dma_start(out=out, in_=in_)
                _dma_i[0] += 1

            xsrc = r3(xt_in)
            # wq + x[i 0:1024] first (Q proj of pair 0 unblocks the PE)
            dma(wq_c[0][:], r3(wq_in)[:, 0])
            dma(xts[0][:, 0:2, :], xsrc[:, 0:2, 0:512])
            dma(wq_c[1][:], r3(wq_in)[:, 1])
            dma(xts[0][:, 2:4, :], xsrc[:, 2:4, 0:512])
            dma(wq_c[2][:], r3(wq_in)[:, 2])
            dma(wq_c[3][:], r3(wq_in)[:, 3])
            dma(relb[:], relb_in[:])
            for chh in range(2):
                dma(xts[1][:, chh * 2:(chh + 1) * 2, :],
                    xsrc[:, chh * 2:(chh + 1) * 2, 512:1024])
            for ch in range(NCH):
                dma(wk[:, ch], r3(wk_in)[:, ch])
            for ch in range(NCH):
                dma(wv[:, ch], r3(wv_in)[:, ch])
            for q in range(2, 4):
                for chh in range(2):
                    dma(xts[q][:, chh * 2:(chh + 1) * 2, :],
                        xsrc[:, chh * 2:(chh + 1) * 2, q * 512:(q + 1) * 512])
            nc.sync.dma_start(out=onesb_t[:], in_=onesb_in[:])
            for ch in range(NCH):
                (nc.sync if ch % 2 == 0 else nc.gpsimd).dma_start(
                    out=wo[:, ch], in_=r3(wo_in)[:, ch])
            nc.gpsimd.dma_start(out=bo[:], in_=bo_in[:])
            nc.gpsimd.partition_broadcast(bo_b[:], bo[:])
            # ones columns of V_aug: contiguous DMA to scratch, strided DVE copy
            for vh in range(2):
                nc.vector.tensor_copy(
                    (vv_a if vh == 0 else vv_b)[:]
                    .rearrange("p j (h e) -> p (j h) e", e=65)[:, :, 64:65],
                    onesb_t[:, vh * NJC * H // 2:(vh + 1) * NJC * H // 2]
                    .rearrange("p (n o) -> p n o", o=1))

            with (
                tc.tile_pool(name="pt", bufs=6) as ptp,
                tc.tile_pool(name="norm", bufs=2) as np_,
                tc.tile_pool(name="ysb", bufs=2) as yp_sb,
            ):
                # ---- head-pair packed Q^T / K^T projections (f32r) ----
                def emit_qk(p, kgs=(0, 1, 2, 3), q=True, sc_copy=False):
                    qt = qt_t0 if p % 2 == 0 else qt_t1
                    ktA = kt_h[(2 * p) % 4]
                    ktB = kt_h[(2 * p + 1) % 4]
                    cols = slice(p * 128, (p + 1) * 128)
                    for g in range(2 if q else 0):
                        q_ps = qkp.tile([128, 512], F32, tag="qk")
                        for ch in range(NCH):
                            nc.tensor.matmul(
                                q_ps[:], wq_c[ch][:, cols],
                                xtv(ch, g * 512, 512),
                                start=(ch == 0), stop=(ch == NCH - 1))
                        nc.vector.tensor_scalar_add(
                            qt[:, g * 512:(g + 1) * 512], q_ps[:],
                            relb[:, p:p + 1])
                    for g in kgs:
                        k_ps = qkp.tile([128, 512], F32, tag="qk")
                        for ch in range(NCH):
                            nc.tensor.matmul(
                                k_ps[:], wk[:, ch, cols], xtv(ch, g * 512, 512),
                                start=(ch == 0), stop=(ch == NCH - 1))
                        ceng = nc.scalar if sc_copy else nc.vector
                        if sc_copy:
                            nc.scalar.copy(
                                ktA[0:64, g * 512:(g + 1) * 512], k_ps[0:64, :])
                            nc.scalar.copy(
                                ktB[64:128, g * 512:(g + 1) * 512],
                                k_ps[64:128, :])
                        else:
                            nc.vector.tensor_copy(
                                ktA[0:64, g * 512:(g + 1) * 512], k_ps[0:64, :])
                            nc.vector.tensor_copy(
                                ktB[64:128, g * 512:(g + 1) * 512],
                                k_ps[64:128, :])

                def emit_v(bi):
                    for jc in range(bi * JB * 2, (bi + 1) * JB * 2):
                        v_ps = qkp.tile([128, 512], F32, tag="qk")
                        for ch in range(NCH):
                            nc.tensor.matmul(
                                v_ps[:], xtv(ch, jc * 128, 128), wv[:, ch],
                                start=(ch == 0), stop=(ch == NCH - 1))
                        nc.vector.tensor_copy(
                            vvt(jc).rearrange("p (h e) -> p h e", e=65)[:, :, 0:64],
                            v_ps[:].rearrange("p (h e) -> p h e", e=64))

                def emit_yproj(ibs):
                    for ib in ibs:
                        y_ps = qkp.tile([128, 512], F32, tag="qk")
                        for hp2 in range(NP):
                            nc.tensor.matmul(
                                y_ps[:],
                                outt_p[hp2][:, ib * 128:(ib + 1) * 128],
                                wo[:, hp2], start=(hp2 == 0),
                                stop=(hp2 == NP - 1))
                        y_sb = yp_sb.tile([128, MODEL], F32, tag="ysb")
                        nc.vector.tensor_tensor(out=y_sb[:], in0=y_ps[:],
                                                in1=bo_b[:],
                                                op=mybir.AluOpType.add)
                        nc.sync.dma_start(
                            out=y_out[ib * 128:(ib + 1) * 128, :], in_=y_sb[:])

                # zero the pad halves of the per-head K tiles once
                for i in range(4):
                    half = slice(64, 128) if i % 2 == 0 else slice(0, 64)
                    nc.gpsimd.memset(kt_h[i][half, :], 0.0)

                emit_qk(0, kgs=(0, 1))

                for h in range(H):
                    hp, hr = h // 2, (h % 2) * 64
                    qt = qt_t0 if hp % 2 == 0 else qt_t1
                    kt = kt_h[h % 4]
                    rows = slice(hr, hr + 64)
                    for ih in range(2):
                        pv_t = pvp.tile([65, 512], F32, tag="pv")
                        isl = slice(ih * 512, (ih + 1) * 512)
                        pts = {}
                        for bj in range(0, NB + 4, 2):
                            for bi in (bj, bj + 1):
                                if bi >= NB:
                                    continue
                                st = stp.tile([128, JB * 512], F32, tag="st")
                                for k in range(JB):
                                    jc = bi * JB + k
                                    nc.tensor.matmul(
                                        st[:, k * 512:(k + 1) * 512],
                                        kt[:, jc * 128:(jc + 1) * 128],
                                        qt[:, isl], start=True, stop=True)
                                pt = ptp.tile([128, JB * 512], BF16, tag="pt")
                                pts[bi] = pt
                                nc.scalar.activation(pt[:], st[:], EXP,
                                                     scale=1.0)
                            # interleave V projection during head 0's first half
                            if h == 0 and ih == 0 and bj < 4:
                                emit_v(bj)
                                emit_v(bj + 1)
                            if h == 0 and ih == 0 and bj == 0:
                                emit_qk(0, kgs=(2, 3), q=False, sc_copy=True)
                            if (ih == 0 and h % 2 == 0 and h + 2 < H
                                    and bj == (2 if h == 0 else 0)):
                                emit_qk(hp + 1, sc_copy=(h == 0))
                            for bi in (bj - 4, bj - 3):
                                if not (0 <= bi < NB):
                                    continue
                                ptb = pts.pop(bi)
                                for k in range(JB):
                                    jc = bi * JB + k
                                    nc.tensor.matmul(
                                        pv_t[:],
                                        vvt(jc)[:, h * 65:(h + 1) * 65],
                                        ptb[:, k * 512:(k + 1) * 512],
                                        start=(jc == 0),
                                        stop=(jc == NJC - 1))
                        den = np_.tile([1, 512], F32, tag="den")
                        nc.vector.tensor_copy(den[:], pv_t[64:65, :])
                        rrow = np_.tile([1, 512], F32, tag="rrow")
                        nc.vector.reciprocal_approx_fast(rrow[:], den[:])
                        rb = np_.tile([64, 512], F32, tag="rb")
                        nc.gpsimd.partition_broadcast(rb[:], rrow[:])
                        nc.vector.tensor_tensor(
                            out=outt_p[hp][rows, isl],
                            in0=pv_t[0:64, :], in1=rb[:],
                            op=mybir.AluOpType.mult)
                        if h == H - 1 and ih == 0:
                            emit_yproj(range(4))

                emit_yproj(range(4, NI // 128))

    nc.compile()
    return nc


def _get_compiled():
    global _COMPILED
    if _COMPILED is None:
        _COMPILED = _build()
    return _COMPILED


def kernel(x, Wq, Wk, Wv, Wo, bo, rel_content_bias, _trace=False):
    from concourse.bass_utils import run_bass_kernel_spmd
    import ml_dtypes

    nc = _get_compiled()

    x = np.asarray(x, dtype=np.float32)
    Wq = np.asarray(Wq, dtype=np.float32)
    Wk = np.asarray(Wk, dtype=np.float32)
    Wv = np.asarray(Wv, dtype=np.float32)
    Wo = np.asarray(Wo, dtype=np.float32)
    bo = np.asarray(bo, dtype=np.float32)
    bias = np.asarray(rel_content_bias, dtype=np.float32).reshape(H, DK)

    Wq_s = (Wq * SCALE).astype(np.float32)
    # relb column p = [bias of head 2p (64) | bias of head 2p+1 (64)]
    relb = bias.reshape(NP, 2 * DK).T.astype(np.float32)  # [128, NP]
    onesb = np.ones((128, NJC * H), ml_dtypes.bfloat16)
    shared = {"wq": Wq_s.astype(np.float16), "wk": Wk.astype(np.float16),
              "wv": Wv.astype(np.float16), "relb": relb,
              "wo": Wo.astype(np.float16), "bo": bo[None, :], "onesb": onesb}

    in_maps = []
    for c in range(8):
        b, half = c // 2, c % 2
        xt = np.ascontiguousarray(x[b].T).astype(np.float16)   # [512, 2048]
        if half:
            xt = np.ascontiguousarray(np.roll(xt, -NI, axis=1))
        in_maps.append({"xt": xt, **shared})

    res = run_bass_kernel_spmd(nc, in_maps, core_ids=list(range(8)),
                               trace=_trace)
    out = np.empty((B, N, MODEL), np.float32)
    for c in range(8):
        b, half = c // 2, c % 2
        out[b, half * NI:(half + 1) * NI, :] = res.results[c]["y"]
    if _trace:
        return out, res
    return out
